# revision 20
# baseline (speedup 1.0000x reference)
"""Trainium2 Bass kernel for nn_ComplexMixture.

Reference:
  output_real[b,n,m] = sum_s w[b,s] * (r[b,s,n]*r[b,s,m] + i[b,s,n]*i[b,s,m])
  output_imag[b,n,m] = sum_s w[b,s] * (i[b,s,n]*r[b,s,m] - r[b,s,n]*i[b,s,m])

Shapes: B=32, S=128, N=256, fp32. w is uniform [0,1) so sqrt(w) is real.

Data-parallel over B across 8 cores, 4 batches/core.

Host-side packing is chosen so every DMA descriptor is >=4KB contiguous
per SBUF partition (DMA efficiency):
  xpack [S, 8 + 2*N*BPC]: per partition s: [sqrt(w).T | -sqrt(w).T | b0:(r|i) | b1:(r|i) | ...]
  out   [BPC, 128, 2, 2, N]: per (b, p): 4KB contiguous [t, c, m] block.
Host reassembles (out_r, out_i) from the device layout.

Per core (S=128 = partition/contraction dim):
  X_all <- 2 DMAs (SP ring: swn+b01, ACT ring: b23)
  warmup: f32r dummy matmuls keep the PE clock un-throttled during loads
  per batch b:
    Y  = sqrt(w_b)[:,None]*X_b    [128,512]  DVE; rounds into matmul dtype
    Yn = -sqrt(w_b)[:,None]*r_b   [128,256]  DVE
    ps_r[:, c*256:+256] = Yr_c.T @ Yr + Yi_c.T @ Yi   (PSUM accumulation, c=0,1)
    ps_i[:, c*256:+256] = Yi_c.T @ Yr + Yn_c.T @ Yi
    O[:, 0:512] = ps_r (DVE copy); O[:, 512:1024] = ps_i (ACT copy)
    one DMA: O -> out[b]  (pure 2-dim AP), alternating SP / GpSimd
"""

import os

import numpy as np

import concourse.bass as bass
import concourse.mybir as mybir
import concourse.tile as tile
from concourse import bacc
from concourse.bass_utils import run_bass_kernel_spmd

B, S, N = 32, 128, 256
NCORES = 8
BPC = B // NCORES  # batches per core
WCOL = 2 * BPC  # swn columns
XCOL = WCOL + 2 * N * BPC

F32 = mybir.dt.float32
# Matmul operand dtype: float32r streams at 1 cycle/row (vs 4 for float32).
MM_DT = mybir.dt.float32r if os.environ.get("CM_MM_F32R", "1") == "1" else F32
N_WARMUP = int(os.environ.get("CM_WARMUP", "14"))

LAST_RESULTS = None  # stashed BassKernelResults for test harness introspection


def build_nc() -> bass.Bass:
    nc = bacc.Bacc(num_swdge_queues=2)
    xin = nc.dram_tensor("xpack", [S, XCOL], F32, kind="ExternalInput")
    out = nc.dram_tensor("out_all", [BPC, 128, 2, 2, N], F32, kind="ExternalOutput")
    half = WCOL + N * BPC  # split point for the two input DMAs

    with tile.TileContext(nc) as tc:
        with (
            tc.tile_pool(name="io", bufs=1) as io_pool,
            tc.tile_pool(name="yp", bufs=BPC) as y_pool,
            tc.tile_pool(name="op", bufs=BPC) as out_pool,
            tc.tile_pool(name="ps", bufs=3, space="PSUM") as ps_pool,
            tc.tile_pool(name="wu", bufs=1, space="PSUM") as wu_pool,
        ):
            # PE warmup: f32r matmuls on scratch data with minimal deps keep
            # the PE HAM clock warm while the input DMAs stream in.
            if N_WARMUP:
                junk = io_pool.tile([S, N], F32, tag="junk", name="junk")
                nc.gpsimd.memset(junk, 1.0)
                junk_r = io_pool.tile([S, N], MM_DT, tag="junkr", name="junk_r")
                nc.vector.tensor_scalar_mul(junk_r, junk, 1.0)
                wups = wu_pool.tile([128, N], F32, tag="wu", name="wups")
                for k in range(N_WARMUP):
                    nc.tensor.matmul(
                        wups, lhsT=junk_r[:, 0:128], rhs=junk_r,
                        start=True, stop=True, skip_group_check=True,
                    )

            X_all = io_pool.tile([S, XCOL], F32, tag="X", name="X_all")
            cut1 = WCOL + 2 * N      # swn + b0
            cut2 = WCOL + 4 * N      # b1
            nc.sync.dma_start(out=X_all[:, 0:cut1], in_=xin[:, 0:cut1])
            nc.gpsimd.dma_start(out=X_all[:, cut1:cut2], in_=xin[:, cut1:cut2])
            nc.scalar.dma_start(out=X_all[:, cut2:XCOL], in_=xin[:, cut2:XCOL])
            sw = X_all[:, 0:BPC]
            nsw = X_all[:, BPC:WCOL]

            for b in range(BPC):
                X = X_all[:, WCOL + b * 2 * N : WCOL + (b + 1) * 2 * N]
                Y = y_pool.tile([S, 2 * N], MM_DT, tag="Y", name=f"Y{b}")
                nc.vector.tensor_scalar_mul(Y, X, sw[:, b : b + 1])
                Yn = y_pool.tile([S, N], MM_DT, tag="Yn", name=f"Yn{b}")
                nc.scalar.activation(
                    out=Yn, in_=X[:, 0:N],
                    func=mybir.ActivationFunctionType.Copy,
                    scale=nsw[:, b : b + 1],
                )

                Yr = Y[:, 0:N]
                Yi = Y[:, N : 2 * N]
                ps_r = ps_pool.tile([128, 2 * N], F32, tag="psR", name=f"psR{b}")
                ps_i = ps_pool.tile([128, 2 * N], F32, tag="psI", name=f"psI{b}")
                for c in range(2):
                    csl = slice(c * 128, c * 128 + 128)
                    osl = slice(c * N, (c + 1) * N)
                    nc.tensor.matmul(ps_r[:, osl], lhsT=Yr[:, csl], rhs=Yr, start=True, stop=False)
                    nc.tensor.matmul(ps_r[:, osl], lhsT=Yi[:, csl], rhs=Yi, start=False, stop=True)
                    nc.tensor.matmul(ps_i[:, osl], lhsT=Yi[:, csl], rhs=Yr, start=True, stop=False)
                    nc.tensor.matmul(ps_i[:, osl], lhsT=Yn[:, csl], rhs=Yi, start=False, stop=True)

                O = out_pool.tile([128, 4 * N], F32, tag="O", name=f"O{b}")
                nc.vector.tensor_copy(O[:, 0 : 2 * N], ps_r)
                nc.scalar.copy(out=O[:, 2 * N : 4 * N], in_=ps_i)
                # out[b, p, t, c, m] <- O[p, (t c m)]; 2-dim AP both sides
                dst = out[b].rearrange("p t c m -> p (t c m)")
                if b == 0:
                    nc.sync.dma_start(out=dst, in_=O)
                elif b == 2:
                    nc.scalar.dma_start(out=dst, in_=O)
                else:
                    nc.gpsimd.dma_start(out=dst, in_=O)
    nc.compile()
    return nc


def kernel(**inputs: np.ndarray):
    global LAST_RESULTS
    r = np.asarray(inputs["input_real"], dtype=np.float32)
    i = np.asarray(inputs["input_imag"], dtype=np.float32)
    w = np.ascontiguousarray(np.asarray(inputs["weight"], dtype=np.float32))
    assert r.shape == (B, S, N) and i.shape == (B, S, N) and w.shape == (B, S)

    # [B, 2, S, N] -> per-core [S, (b t n)] batch-major blocks
    xin = np.stack([r, i], axis=1)
    sws = np.sqrt(w)  # [B, S]

    in_maps = []
    for c in range(NCORES):
        sl = slice(c * BPC, (c + 1) * BPC)
        xc = np.transpose(xin[sl], (2, 0, 1, 3)).reshape(S, 2 * N * BPC)
        xpack = np.concatenate([sws[sl].T, -sws[sl].T, xc], axis=1)
        in_maps.append({"xpack": np.ascontiguousarray(xpack)})

    nc = build_nc()
    res = run_bass_kernel_spmd(nc, in_maps, core_ids=list(range(NCORES)))
    LAST_RESULTS = res

    out_all = np.concatenate(
        [res.results[c]["out_all"] for c in range(NCORES)], axis=0
    )  # [B, 128, 2, 2, N]
    # out[t][b, c*128+p, m] = out_all[b, p, t, c, m]
    out_all = np.transpose(out_all, (2, 0, 3, 1, 4)).reshape(2, B, N, N)
    return (np.ascontiguousarray(out_all[0]), np.ascontiguousarray(out_all[1]))
